# revision 10
# baseline (speedup 1.0000x reference)
"""Trainium2 Bass kernel for nn_Affine_Module_Abla (VN-style affine + VNLeakyReLU).

Math (per batch b, point n, channel d<128, with u=J[...,0], v=J[...,1], X):
  b1 = u/||u||; w = v - (u.v/||u||^2) u; b2 = w/||w||; b3 = b1 x b2
  a = (b1.X, b2.X, b3.X)                                  # [128,3] per n
  x = M3 @ a   (M3 = A+B+C)                               # [256,3] per n
  d = Wdir @ x = (Wdir@M3) @ a = WM @ a
  out = x - 0.8*min(dot,0)/(dns+eps)*d,  dot = x.d over i, dns = d.d over i

Sharding: batch B=8 -> one batch per NeuronCore (8 cores).
Host precomputes SoA planes [9,128,N] (u0..u2,v0..v2,x0..x2) per core, plus
weight tiles (WM pre-scaled by 1/64 so the fp16 epilogue stays in range);
device does everything else.

Device strategy:
 - stage A in fp32 on DVE/ACT; 3-term dot reductions (p=u.X, wX=w.X,
   det=(u x w).X) are accumulated on the PE via identity matmuls into PSUM.
 - Y/D matmuls in f32r (full PE speed).
 - epilogue in fp16 (error-checked: absmax-scale contribution ~1e-3), final
   subtract against exact f32 Y from PSUM.
"""

import numpy as np

_B, _N, _D, _F = 8, 4096, 128, 256
_T = 512
_NCHUNK = _N // _T
_EPS = 1e-6  # VNLeakyReLU eps
_DSC = 64.0  # d is computed scaled by 1/64 for fp16 range
_EPS_S = _EPS / (_DSC * _DSC)

_cache = {}


def _build_nc():
    import concourse.bacc as bacc
    import concourse.mybir as mybir
    from concourse.tile import TileContext

    f32 = mybir.dt.float32
    f32r = mybir.dt.float32r
    f16 = mybir.dt.float16
    Alu = mybir.AluOpType

    nc = bacc.Bacc("TRN2", target_bir_lowering=False)
    UVX = nc.declare_dram_parameter("uvx", [9, _D, _N], f32, isOutput=False)
    WY = nc.declare_dram_parameter("wy", [2, _D, 128], f32, isOutput=False)
    WD = nc.declare_dram_parameter("wd", [2, _D, 128], f32, isOutput=False)
    EYE = nc.declare_dram_parameter("eye", [_D, 128], f32, isOutput=False)
    OUT = nc.declare_dram_parameter("out", [_F, 3, _N], f32, isOutput=True)

    with TileContext(nc) as tc:
        with (
            tc.tile_pool(name="w", bufs=1) as wp,
            tc.tile_pool(name="io", bufs=2) as iop,
            tc.tile_pool(name="st", bufs=1) as stp,
            tc.tile_pool(name="ab", bufs=2) as abp,
            tc.tile_pool(name="ep", bufs=1) as epp,
            tc.tile_pool(name="ob", bufs=3) as obp,
            tc.tile_pool(name="ps", bufs=1, space="PSUM") as psp,
        ):
            # weights: DMA f32 then round to f32r via DVE copies (one-time)
            wy, wd = [], []
            for h in range(2):
                w1 = wp.tile([_D, 128], f32, tag=f"wy{h}", name=f"wy{h}")
                nc.sync.dma_start(out=w1[:], in_=WY[h])
                w2 = wp.tile([_D, 128], f32, tag=f"wd{h}", name=f"wd{h}")
                nc.sync.dma_start(out=w2[:], in_=WD[h])
                wy.append(w1)
                wd.append(w2)
            eyef = wp.tile([_D, 128], f32, tag="eyef", name="eyef")
            nc.sync.dma_start(out=eyef[:], in_=EYE[:])
            eye = wp.tile([_D, 128], f32r, tag="eye", name="eye")
            nc.vector.tensor_copy(eye[:], eyef[:])
            eye16 = wp.tile([_D, 128], f16, tag="eye16", name="eye16")
            nc.vector.tensor_copy(eye16[:], eyef[:])

            def T_(tag, pool=stp, bufs=None, dt=f32):
                return pool.tile([_D, _T], dt, tag=tag, name=tag, bufs=bufs)

            def tt(o, a, b, op):
                nc.vector.tensor_tensor(o[:], a[:], b[:], op)

            for ci in range(_NCHUNK):
                sl = slice(ci * _T, (ci + 1) * _T)
                pl = []
                for j in range(9):
                    tin = iop.tile([_D, _T], f32, tag=f"in{j}", name=f"in{j}")
                    nc.sync.dma_start(out=tin[:], in_=UVX[j, :, sl])
                    pl.append(tin)
                u = pl[0:3]
                v = pl[3:6]
                x = pl[6:9]

                def pe_sum3(tag_ps, prods, ey=None, ptag="psA", pbufs=2):
                    ps = psp.tile([128, _T], f32, tag=ptag, name=tag_ps, bufs=pbufs)
                    for j in range(3):
                        nc.tensor.matmul(
                            ps[:], (ey or eye)[:], prods[j][:],
                            start=(j == 0), stop=(j == 2)
                        )
                    return ps

                # c = u.v, nu2 = u.u : exact fp32 products + fp32 PE-sums
                cm = []
                for j in range(3):
                    m = T_("cm", bufs=3)
                    tt(m, u[j], v[j], Alu.mult)
                    cm.append(m)
                cuv = pe_sum3("cuv_ps", cm, ey=eyef)

                squ = []
                for j in range(3):
                    sq_ = T_("squ", bufs=3)
                    nc.scalar.square(sq_[:], u[j][:])
                    squ.append(sq_)
                nu2 = pe_sum3("nu2_ps", squ, ey=eyef)

                inv_u = T_("inv_u")
                rscr = T_("rscr")
                nc.vector.reciprocal_approx_accurate(
                    out=inv_u[:], in_=nu2[:], scratch=rscr[:]
                )
                t_ = T_("t")
                tt(t_, cuv, inv_u, Alu.mult)
                w = []
                for j in range(3):
                    tu = T_("ta")
                    tt(tu, t_, u[j], Alu.mult)
                    wj = T_(f"w{j}")
                    tt(wj, v[j], tu, Alu.subtract)
                    w.append(wj)

                sqw = []
                for j in range(3):
                    sq_ = T_("sqw", bufs=3, dt=f32r)
                    nc.scalar.square(sq_[:], w[j][:])
                    sqw.append(sq_)
                nw2_ps = pe_sum3("nw2_ps", sqw)
                nw2 = T_("nw2")
                nc.vector.tensor_scalar_max(nw2[:], nw2_ps[:], 1e-30)
                inv_w = T_("inv_w")
                nc.vector.reciprocal_approx_fast(out=inv_w[:], in_=nw2[:])
                rs_u = T_("rs_u")
                nc.scalar.sqrt(rs_u[:], inv_u[:])
                rs_w = T_("rs_w")
                nc.scalar.sqrt(rs_w[:], inv_w[:])

                # p = u.X, wX = w.X, det = (u x w).X  -- products on DVE (f32r
                # out), 3-term sums accumulated on PE via identity matmuls.
                pm = []
                for j in range(3):
                    m = T_(f"pm{j}", dt=f32r)
                    tt(m, u[j], x[j], Alu.mult)
                    pm.append(m)
                p_ps = pe_sum3("p_ps", pm)

                wm = []
                for j in range(3):
                    m = T_(f"wm{j}", dt=f32r)
                    tt(m, w[j], x[j], Alu.mult)
                    wm.append(m)
                w_ps = pe_sum3("w_ps", wm)

                dm = []
                for k, (i, j) in enumerate([(1, 2), (2, 0), (0, 1)]):
                    c1 = T_("cra")
                    tt(c1, u[i], w[j], Alu.mult)
                    c2 = T_("crb")
                    tt(c2, u[j], w[i], Alu.mult)
                    cr = T_("cr")
                    tt(cr, c1, c2, Alu.subtract)
                    m = T_(f"dm{k}", dt=f32r)
                    tt(m, cr, x[k], Alu.mult)
                    dm.append(m)
                d_ps = pe_sum3("d_ps", dm)

                # a_i in f32r (rounded on DVE write), straight from PSUM sums
                a0 = abp.tile([_D, _T], f32, tag="a0", name="a0")
                tt(a0, p_ps, rs_u, Alu.mult)
                a1 = abp.tile([_D, _T], f32, tag="a1", name="a1")
                tt(a1, w_ps, rs_w, Alu.mult)
                s2c = T_("s2c")
                tt(s2c, rs_u, rs_w, Alu.mult)
                a2 = abp.tile([_D, _T], f32, tag="a2", name="a2")
                tt(a2, d_ps, s2c, Alu.mult)
                aa = [a0, a1, a2]

                for h in range(2):
                    px = [
                        psp.tile([128, _T], f32, tag="mm", name=f"px{i}", bufs=4)
                        for i in range(3)
                    ]
                    pd = [
                        psp.tile([128, _T], f32, tag="mm", name=f"pd{i}", bufs=4)
                        for i in range(3)
                    ]
                    for i in range(3):
                        nc.tensor.matmul(
                            px[i][:], wy[h][:], aa[i][:], start=True, stop=True
                        )
                    for i in range(3):
                        nc.tensor.matmul(
                            pd[i][:], wd[h][:], aa[i][:], start=True, stop=True
                        )

                    # fp16 epilogue; d was computed scaled by 1/64
                    x16 = []
                    d16 = []
                    for i in range(3):
                        xt = epp.tile([128, _T], f16, tag=f"x16_{i}", name=f"x16_{i}", bufs=2)
                        nc.scalar.copy(xt[:], px[i][:])
                        x16.append(xt)
                        dt_ = epp.tile([128, _T], f16, tag=f"d16_{i}", name=f"d16_{i}", bufs=2)
                        nc.scalar.copy(dt_[:], pd[i][:])
                        d16.append(dt_)

                    pr = []
                    for i in range(3):
                        pr_ = epp.tile([128, _T], f16, tag="pr", name="pr", bufs=3)
                        tt(pr_, x16[i], d16[i], Alu.mult)
                        pr.append(pr_)
                    dot_ps = pe_sum3("dot_ps", pr, ey=eye16, ptag="psE", pbufs=2)

                    dq = []
                    for i in range(3):
                        dq_ = epp.tile([128, _T], f16, tag="dq", name="dq", bufs=3)
                        nc.scalar.square(dq_[:], d16[i][:])
                        dq.append(dq_)
                    dns_ps = pe_sum3("dns_ps", dq, ey=eye16, ptag="psE", pbufs=2)

                    denom = epp.tile([128, _T], f32, tag="denom", name="denom")
                    nc.vector.tensor_scalar(
                        out=denom[:],
                        in0=dns_ps[:],
                        scalar1=_EPS_S,
                        scalar2=1.25,
                        op0=Alu.add,
                        op1=Alu.mult,
                    )
                    inv = epp.tile([128, _T], f32, tag="inv", name="inv")
                    nc.vector.reciprocal_approx_fast(out=inv[:], in_=denom[:])
                    s_ = epp.tile([128, _T], f16, tag="s", name="s")
                    nc.vector.scalar_tensor_tensor(
                        s_[:], dot_ps[:], 0.0, inv[:], Alu.min, Alu.mult
                    )

                    for i in range(3):
                        g = epp.tile([128, _T], f16, tag="g", name="g")
                        tt(g, s_, d16[i], Alu.mult)
                        o = obp.tile([128, _T], f16, tag=f"o{i}", name=f"o{i}")
                        tt(o, x16[i], g, Alu.subtract)
                        nc.gpsimd.dma_start(
                            out=OUT[h * 128 : (h + 1) * 128, i, sl], in_=o[:]
                        )

    nc.compile()
    return nc


def _get_nc():
    if "nc" not in _cache:
        _cache["nc"] = _build_nc()
    return _cache["nc"]


def _host_prep(X, J, Amat, Bmat, Cmat, Wdir):
    X = np.ascontiguousarray(np.asarray(X, dtype=np.float32))
    J = np.ascontiguousarray(np.asarray(J, dtype=np.float32))
    Amat = np.asarray(Amat, dtype=np.float32)
    Bmat = np.asarray(Bmat, dtype=np.float32)
    Cmat = np.asarray(Cmat, dtype=np.float32)
    Wdir = np.asarray(Wdir, dtype=np.float32)

    M3 = Amat + Bmat + Cmat  # [F, D]
    WM = (Wdir @ M3) / _DSC  # [F, D], pre-scaled for fp16 epilogue range
    WY = np.ascontiguousarray(np.stack([M3[:128, :].T, M3[128:, :].T]))  # [2, D, 128]
    WD = np.ascontiguousarray(np.stack([WM[:128, :].T, WM[128:, :].T]))
    EYE = np.eye(_D, dtype=np.float32)

    in_maps = []
    for b in range(_B):
        uvx = np.empty((9, _D, _N), dtype=np.float32)
        Jt = J[b].transpose(3, 2, 1, 0)  # [2, 3, D, N]
        uvx[0:3] = Jt[0]
        uvx[3:6] = Jt[1]
        uvx[6:9] = X[b].transpose(2, 1, 0)  # [3, D, N]
        in_maps.append({"uvx": uvx, "wy": WY, "wd": WD, "eye": EYE})
    return in_maps


def run(X, J, Amat, Bmat, Cmat, Wdir, device=None, trace=False):
    from concourse.bass_utils import run_bass_kernel_spmd

    nc = _get_nc()
    in_maps = _host_prep(X, J, Amat, Bmat, Cmat, Wdir)
    res = run_bass_kernel_spmd(nc, in_maps, list(range(_B)), trace=trace)
    out = np.stack([res.results[b]["out"] for b in range(_B)], axis=0)
    return out, res


def kernel(X, J, Amat, Bmat, Cmat, Wdir, device=None):
    out, _ = run(X, J, Amat, Bmat, Cmat, Wdir, device)
    return out


# revision 11
# speedup vs baseline: 1.1412x; 1.1412x over previous
"""Trainium2 Bass kernel for nn_Affine_Module_Abla (VN-style affine + VNLeakyReLU).

Math (per batch b, point n, channel d<128, with u=J[...,0], v=J[...,1], X):
  b1 = u/||u||; w = v - (u.v/||u||^2) u; b2 = w/||w||; b3 = b1 x b2
  a = (b1.X, b2.X, b3.X)                                  # [128,3] per n
  x = M3 @ a   (M3 = A+B+C)                               # [256,3] per n
  d = Wdir @ x = (Wdir@M3) @ a = WM @ a
  out = x - 0.8*min(dot,0)/(dns+eps)*d,  dot = x.d over i, dns = d.d over i

Sharding: batch B=8 -> one batch per NeuronCore (8 cores).
Host precomputes SoA planes [9,128,N] (u0..u2,v0..v2,x0..x2) per core, plus
weight tiles (WM pre-scaled by 1/64 so the fp16 epilogue stays in range);
device does everything else.

Device strategy:
 - stage A in fp32 on DVE/ACT; 3-term dot reductions (p=u.X, wX=w.X,
   det=(u x w).X) are accumulated on the PE via identity matmuls into PSUM.
 - Y/D matmuls in f32r (full PE speed).
 - epilogue in fp16 (error-checked: absmax-scale contribution ~1e-3), final
   subtract against exact f32 Y from PSUM.
"""

import numpy as np

_B, _N, _D, _F = 8, 4096, 128, 256
_T = 512
_NCHUNK = _N // _T
_EPS = 1e-6  # VNLeakyReLU eps
_DSC = 64.0  # d is computed scaled by 1/64 for fp16 range
_EPS_S = _EPS / (_DSC * _DSC)

_cache = {}


def _build_nc():
    import concourse.bacc as bacc
    import concourse.mybir as mybir
    from concourse.tile import TileContext

    f32 = mybir.dt.float32
    f32r = mybir.dt.float32r
    f16 = mybir.dt.float16
    Alu = mybir.AluOpType

    nc = bacc.Bacc("TRN2", target_bir_lowering=False)
    UVX = nc.declare_dram_parameter("uvx", [9, _D, _N], f32, isOutput=False)
    WY = nc.declare_dram_parameter("wy", [2, _D, 128], f32, isOutput=False)
    WD = nc.declare_dram_parameter("wd", [2, _D, 128], f32, isOutput=False)
    EYE = nc.declare_dram_parameter("eye", [_D, 128], f32, isOutput=False)
    OUT = nc.declare_dram_parameter("out", [_F, 3, _N], f32, isOutput=True)

    with TileContext(nc) as tc:
        with (
            tc.tile_pool(name="w", bufs=1) as wp,
            tc.tile_pool(name="io", bufs=2) as iop,
            tc.tile_pool(name="st", bufs=1) as stp,
            tc.tile_pool(name="ab", bufs=2) as abp,
            tc.tile_pool(name="ep", bufs=1) as epp,
            tc.tile_pool(name="ob", bufs=3) as obp,
            tc.tile_pool(name="ps", bufs=1, space="PSUM") as psp,
        ):
            # weights: DMA f32 then round to f32r via DVE copies (one-time)
            wy, wd = [], []
            for h in range(2):
                w1 = wp.tile([_D, 128], f32, tag=f"wy{h}", name=f"wy{h}")
                nc.sync.dma_start(out=w1[:], in_=WY[h])
                w2 = wp.tile([_D, 128], f32, tag=f"wd{h}", name=f"wd{h}")
                nc.sync.dma_start(out=w2[:], in_=WD[h])
                wy.append(w1)
                wd.append(w2)
            eyef = wp.tile([_D, 128], f32, tag="eyef", name="eyef")
            nc.sync.dma_start(out=eyef[:], in_=EYE[:])
            eye = wp.tile([_D, 128], f32r, tag="eye", name="eye")
            nc.vector.tensor_copy(eye[:], eyef[:])
            eye16 = wp.tile([_D, 128], f16, tag="eye16", name="eye16")
            nc.vector.tensor_copy(eye16[:], eyef[:])

            def T_(tag, pool=stp, bufs=None, dt=f32):
                return pool.tile([_D, _T], dt, tag=tag, name=tag, bufs=bufs)

            def tt(o, a, b, op):
                nc.vector.tensor_tensor(o[:], a[:], b[:], op)

            for ci in range(_NCHUNK):
                sl = slice(ci * _T, (ci + 1) * _T)
                pl = []
                for j in range(9):
                    tin = iop.tile([_D, _T], f32, tag=f"in{j}", name=f"in{j}")
                    nc.sync.dma_start(out=tin[:], in_=UVX[j, :, sl])
                    pl.append(tin)
                u = pl[0:3]
                v = pl[3:6]
                x = pl[6:9]

                def pe_sum3(tag_ps, prods, ey=None, ptag="psA", pbufs=2):
                    ps = psp.tile([128, _T], f32, tag=ptag, name=tag_ps, bufs=pbufs)
                    for j in range(3):
                        nc.tensor.matmul(
                            ps[:], (ey or eye)[:], prods[j][:],
                            start=(j == 0), stop=(j == 2)
                        )
                    return ps

                # c = u.v (DVE), nu2 = u.u (ACT squares + DVE adds)
                ta = T_("ta")
                tt(ta, u[0], v[0], Alu.mult)
                tb = T_("tb")
                tt(tb, u[1], v[1], Alu.mult)
                cuv = T_("cuv")
                tt(cuv, ta, tb, Alu.add)
                ta2 = T_("ta")
                tt(ta2, u[2], v[2], Alu.mult)
                tt(cuv, cuv, ta2, Alu.add)

                sq1 = T_("sq1", bufs=2)
                nc.scalar.square(sq1[:], u[0][:])
                sq2 = T_("sq2", bufs=2)
                nc.scalar.square(sq2[:], u[1][:])
                nu2 = T_("nu2")
                tt(nu2, sq1, sq2, Alu.add)
                sq3 = T_("sq1", bufs=2)
                nc.scalar.square(sq3[:], u[2][:])
                tt(nu2, nu2, sq3, Alu.add)

                inv_u = T_("inv_u")
                rscr = T_("rscr")
                nc.vector.reciprocal_approx_accurate(
                    out=inv_u[:], in_=nu2[:], scratch=rscr[:]
                )
                t_ = T_("t")
                tt(t_, cuv, inv_u, Alu.mult)
                w = []
                for j in range(3):
                    tu = T_("ta")
                    tt(tu, t_, u[j], Alu.mult)
                    wj = T_(f"w{j}")
                    tt(wj, v[j], tu, Alu.subtract)
                    w.append(wj)

                sqw = []
                for j in range(3):
                    sq_ = T_("sqw", bufs=3, dt=f32r)
                    nc.scalar.square(sq_[:], w[j][:])
                    sqw.append(sq_)
                nw2_ps = pe_sum3("nw2_ps", sqw)
                nw2 = T_("nw2")
                nc.vector.tensor_scalar_max(nw2[:], nw2_ps[:], 1e-30)
                inv_w = T_("inv_w")
                nc.vector.reciprocal_approx_fast(out=inv_w[:], in_=nw2[:])
                rs_u = T_("rs_u")
                nc.scalar.sqrt(rs_u[:], inv_u[:])
                rs_w = T_("rs_w")
                nc.scalar.sqrt(rs_w[:], inv_w[:])

                # p = u.X, wX = w.X, det = (u x w).X  -- products on DVE (f32r
                # out), 3-term sums accumulated on PE via identity matmuls.
                pm = []
                for j in range(3):
                    m = T_(f"pm{j}", dt=f32r)
                    tt(m, u[j], x[j], Alu.mult)
                    pm.append(m)
                p_ps = pe_sum3("p_ps", pm)

                wm = []
                for j in range(3):
                    m = T_(f"wm{j}", dt=f32r)
                    tt(m, w[j], x[j], Alu.mult)
                    wm.append(m)
                w_ps = pe_sum3("w_ps", wm)

                dm = []
                for k, (i, j) in enumerate([(1, 2), (2, 0), (0, 1)]):
                    c1 = T_("cra")
                    tt(c1, u[i], w[j], Alu.mult)
                    c2 = T_("crb")
                    tt(c2, u[j], w[i], Alu.mult)
                    cr = T_("cr")
                    tt(cr, c1, c2, Alu.subtract)
                    m = T_(f"dm{k}", dt=f32r)
                    tt(m, cr, x[k], Alu.mult)
                    dm.append(m)
                d_ps = pe_sum3("d_ps", dm)

                # a_i in f32r (rounded on DVE write), straight from PSUM sums
                a0 = abp.tile([_D, _T], f32, tag="a0", name="a0")
                tt(a0, p_ps, rs_u, Alu.mult)
                a1 = abp.tile([_D, _T], f32, tag="a1", name="a1")
                tt(a1, w_ps, rs_w, Alu.mult)
                s2c = T_("s2c")
                tt(s2c, rs_u, rs_w, Alu.mult)
                a2 = abp.tile([_D, _T], f32, tag="a2", name="a2")
                tt(a2, d_ps, s2c, Alu.mult)
                aa = [a0, a1, a2]

                for h in range(2):
                    px = [
                        psp.tile([128, _T], f32, tag="mm", name=f"px{i}", bufs=4)
                        for i in range(3)
                    ]
                    pd = [
                        psp.tile([128, _T], f32, tag="mm", name=f"pd{i}", bufs=4)
                        for i in range(3)
                    ]
                    for i in range(3):
                        nc.tensor.matmul(
                            px[i][:], wy[h][:], aa[i][:], start=True, stop=True
                        )
                    for i in range(3):
                        nc.tensor.matmul(
                            pd[i][:], wd[h][:], aa[i][:], start=True, stop=True
                        )

                    # fp16 epilogue; d was computed scaled by 1/64
                    x16 = []
                    d16 = []
                    for i in range(3):
                        xt = epp.tile([128, _T], f16, tag=f"x16_{i}", name=f"x16_{i}", bufs=2)
                        nc.scalar.copy(xt[:], px[i][:])
                        x16.append(xt)
                        dt_ = epp.tile([128, _T], f16, tag=f"d16_{i}", name=f"d16_{i}", bufs=2)
                        nc.scalar.copy(dt_[:], pd[i][:])
                        d16.append(dt_)

                    pr = []
                    for i in range(3):
                        pr_ = epp.tile([128, _T], f16, tag="pr", name="pr", bufs=3)
                        tt(pr_, x16[i], d16[i], Alu.mult)
                        pr.append(pr_)
                    dot_ps = pe_sum3("dot_ps", pr, ey=eye16, ptag="psE", pbufs=2)

                    dq = []
                    for i in range(3):
                        dq_ = epp.tile([128, _T], f16, tag="dq", name="dq", bufs=3)
                        nc.scalar.square(dq_[:], d16[i][:])
                        dq.append(dq_)
                    dns_ps = pe_sum3("dns_ps", dq, ey=eye16, ptag="psE", pbufs=2)

                    denom = epp.tile([128, _T], f32, tag="denom", name="denom")
                    nc.vector.tensor_scalar(
                        out=denom[:],
                        in0=dns_ps[:],
                        scalar1=_EPS_S,
                        scalar2=1.25,
                        op0=Alu.add,
                        op1=Alu.mult,
                    )
                    inv = epp.tile([128, _T], f32, tag="inv", name="inv")
                    nc.vector.reciprocal_approx_fast(out=inv[:], in_=denom[:])
                    s_ = epp.tile([128, _T], f16, tag="s", name="s")
                    nc.vector.scalar_tensor_tensor(
                        s_[:], dot_ps[:], 0.0, inv[:], Alu.min, Alu.mult
                    )

                    for i in range(3):
                        g = epp.tile([128, _T], f16, tag="g", name="g")
                        tt(g, s_, d16[i], Alu.mult)
                        o = obp.tile([128, _T], f16, tag=f"o{i}", name=f"o{i}")
                        tt(o, x16[i], g, Alu.subtract)
                        nc.gpsimd.dma_start(
                            out=OUT[h * 128 : (h + 1) * 128, i, sl], in_=o[:]
                        )

    nc.compile()
    return nc


def _get_nc():
    if "nc" not in _cache:
        _cache["nc"] = _build_nc()
    return _cache["nc"]


def _host_prep(X, J, Amat, Bmat, Cmat, Wdir):
    X = np.ascontiguousarray(np.asarray(X, dtype=np.float32))
    J = np.ascontiguousarray(np.asarray(J, dtype=np.float32))
    Amat = np.asarray(Amat, dtype=np.float32)
    Bmat = np.asarray(Bmat, dtype=np.float32)
    Cmat = np.asarray(Cmat, dtype=np.float32)
    Wdir = np.asarray(Wdir, dtype=np.float32)

    M3 = Amat + Bmat + Cmat  # [F, D]
    WM = (Wdir @ M3) / _DSC  # [F, D], pre-scaled for fp16 epilogue range
    WY = np.ascontiguousarray(np.stack([M3[:128, :].T, M3[128:, :].T]))  # [2, D, 128]
    WD = np.ascontiguousarray(np.stack([WM[:128, :].T, WM[128:, :].T]))
    EYE = np.eye(_D, dtype=np.float32)

    in_maps = []
    for b in range(_B):
        uvx = np.empty((9, _D, _N), dtype=np.float32)
        Jt = J[b].transpose(3, 2, 1, 0)  # [2, 3, D, N]
        uvx[0:3] = Jt[0]
        uvx[3:6] = Jt[1]
        uvx[6:9] = X[b].transpose(2, 1, 0)  # [3, D, N]
        in_maps.append({"uvx": uvx, "wy": WY, "wd": WD, "eye": EYE})
    return in_maps


def run(X, J, Amat, Bmat, Cmat, Wdir, device=None, trace=False):
    from concourse.bass_utils import run_bass_kernel_spmd

    nc = _get_nc()
    in_maps = _host_prep(X, J, Amat, Bmat, Cmat, Wdir)
    res = run_bass_kernel_spmd(nc, in_maps, list(range(_B)), trace=trace)
    out = np.stack([res.results[b]["out"] for b in range(_B)], axis=0)
    return out, res


def kernel(X, J, Amat, Bmat, Cmat, Wdir, device=None):
    out, _ = run(X, J, Amat, Bmat, Cmat, Wdir, device)
    return out
